# revision 29
# baseline (speedup 1.0000x reference)
"""3x3 median filter (reflect padding) on Trainium2, 8-core data parallel.

Input  x: (4, 3, 1024, 1024) float32
Output  : (4, 3, 1024, 1024) float32  (Kornia MedianBlur semantics)

Strategy (v9):
  - Host: cast to fp16 (tolerance 2e-2 >> fp16 eps), reflect-pad H/W by 1
    -> (12, 1026, 1026); shard H across 8 cores: core k gets padded rows
    [128k, 128k+130).
  - Device (per core): images processed in flat groups [2,5,5] along the
    free dim. All 18 min/max ops of the sorted-column median network run
    on the Vector engine as flat contiguous fp16 streams (2 elem/cycle/
    lane — the hard TT cap). Horizontal shifts index the flat stream;
    positions that straddle an image boundary compute garbage that lands
    in lanes the output DMA never reads.
  - DMA: T1 loads through the scalar engine's initiator queue (a single
    queue's descriptor path caps at ~130 GB/s). Small first group
    minimizes the fill before compute starts; the last group's final
    ops + stores are per-image so the output drains during compute.
  - 12 SBUF slots: T0/T1/T2 double-buffered, temps single-buffered with
    in-place reuse (DVE program order serializes them anyway).
"""

import sys

sys.path.insert(0, "/opt/trn_rl_repo")

import numpy as np

B, C, H, W = 4, 3, 1024, 1024
NIMG = B * C            # 12
NCORES = 8
ROWS_PER_CORE = H // NCORES   # 128
WP = W + 2              # 1026 padded width
HP_CORE = ROWS_PER_CORE + 2   # 130 padded rows per core
GROUPS = [2, 4, 6]      # images per flat group: small first group minimizes
                        # the DMA fill before compute starts; each group's
                        # loads hide under the previous group's compute
GMAX = max(GROUPS)
LPAD = GMAX * WP + 8    # slack so +1/+2 shifted reads stay in bounds

_PROGRAM = None
LAST_RESULT = None


def _build_program():
    import concourse.bacc as bacc
    import concourse.tile as tile
    import concourse.mybir as mybir
    from contextlib import ExitStack

    f16 = mybir.dt.float16
    mn = mybir.AluOpType.min
    mx = mybir.AluOpType.max

    nc = bacc.Bacc("TRN2", target_bir_lowering=False, debug=False,
                   num_devices=NCORES)
    x = nc.dram_tensor("x", [NIMG, HP_CORE, WP], f16, kind="ExternalInput").ap()
    y = nc.dram_tensor("y", [NIMG, ROWS_PER_CORE, W], f16,
                       kind="ExternalOutput").ap()

    P = ROWS_PER_CORE  # 128 partitions

    with tile.TileContext(nc) as tc, ExitStack() as ctx:
        iop = ctx.enter_context(tc.tile_pool(name="io", bufs=2))
        tp = ctx.enter_context(tc.tile_pool(name="tmp", bufs=1))
        tt = nc.vector.tensor_tensor

        i0 = 0
        for gi, G in enumerate(GROUPS):
            L = G * WP
            last = gi == len(GROUPS) - 1
            T0 = iop.tile([P, LPAD], f16, tag="T0")
            T1 = iop.tile([P, LPAD], f16, tag="T1")
            T2 = iop.tile([P, LPAD], f16, tag="T2")
            # T1 goes through the scalar engine's DMA queue so the first
            # two loads (all that the first op needs) transfer in parallel
            nc.sync.dma_start(T0[:, 0:L], x[i0:i0 + G, 0:P, :].transpose([1, 0, 2]))
            nc.scalar.dma_start(T1[:, 0:L], x[i0:i0 + G, 1:P + 1, :].transpose([1, 0, 2]))
            nc.sync.dma_start(T2[:, 0:L], x[i0:i0 + G, 2:P + 2, :].transpose([1, 0, 2]))

            # vertical sort3 of rows: lo/mid/hi per column (6 flat ops)
            m = tp.tile([P, LPAD], f16, tag="m")
            M = tp.tile([P, LPAD], f16, tag="M")
            lo = tp.tile([P, LPAD], f16, tag="lo")
            mm = tp.tile([P, LPAD], f16, tag="mm")
            tt(m[:, 0:L], T0[:, 0:L], T1[:, 0:L], op=mn)
            tt(M[:, 0:L], T0[:, 0:L], T1[:, 0:L], op=mx)
            tt(lo[:, 0:L], m[:, 0:L], T2[:, 0:L], op=mn)
            tt(mm[:, 0:L], M[:, 0:L], T2[:, 0:L], op=mn)
            hi = M
            tt(hi[:, 0:L], M[:, 0:L], T2[:, 0:L], op=mx)
            mid = mm
            tt(mid[:, 0:L], m[:, 0:L], mm[:, 0:L], op=mx)

            # horizontal merge, all flat length-L streams; image-boundary
            # positions are garbage in never-read lanes. T tiles double
            # as scratch (dead after the vertical stage).
            # op order interleaves the independent A/B/C chains so back-to-
            # back DVE instructions rarely depend on their predecessor (the
            # engine then overlaps the post-op pipe drain with the next op)
            pm = T2
            pM = m
            pa = T0
            pc = T1
            tt(pm[:, 0:L], mid[:, 0:L], mid[:, 1:L + 1], op=mn)
            tt(pM[:, 0:L], mid[:, 0:L], mid[:, 1:L + 1], op=mx)
            tt(pa[:, 0:L], lo[:, 0:L], lo[:, 1:L + 1], op=mx)
            tt(pc[:, 0:L], hi[:, 0:L], hi[:, 1:L + 1], op=mn)
            t2 = m
            A = T0
            Cm = T1
            tt(t2[:, 0:L], pM[:, 0:L], mid[:, 2:L + 2], op=mn)
            tt(A[:, 0:L], pa[:, 0:L], lo[:, 2:L + 2], op=mx)
            tt(Cm[:, 0:L], pc[:, 0:L], hi[:, 2:L + 2], op=mn)
            Bm = T2
            tt(Bm[:, 0:L], pm[:, 0:L], t2[:, 0:L], op=mx)

            m1 = lo
            M1 = mm
            tt(M1[:, 0:L], A[:, 0:L], Bm[:, 0:L], op=mx)
            tt(m1[:, 0:L], A[:, 0:L], Bm[:, 0:L], op=mn)
            t3 = M1
            tt(t3[:, 0:L], M1[:, 0:L], Cm[:, 0:L], op=mn)
            out = iop.tile([P, LPAD], f16, tag="out")
            if last:
                # per-image final op + store so the output overlaps compute
                for j in range(G):
                    s = j * WP
                    tt(out[:, s:s + W], m1[:, s:s + W], t3[:, s:s + W], op=mx)
                    nc.sync.dma_start(y[i0 + j], out[:, s:s + W])
            else:
                tt(out[:, 0:L], m1[:, 0:L], t3[:, 0:L], op=mx)
                for j in range(G):
                    nc.sync.dma_start(y[i0 + j], out[:, j * WP:j * WP + W])
            i0 += G

    nc.compile()
    return nc


def _get_program():
    global _PROGRAM
    if _PROGRAM is None:
        _PROGRAM = _build_program()
    return _PROGRAM


def kernel(x):
    global LAST_RESULT
    from concourse.bass_utils import run_bass_kernel_spmd
    import os

    x = np.asarray(x, dtype=np.float32)
    xp = np.pad(x.reshape(NIMG, H, W), ((0, 0), (1, 1), (1, 1)),
                mode="reflect").astype(np.float16)
    in_maps = [
        {"x": np.ascontiguousarray(
            xp[:, ROWS_PER_CORE * k: ROWS_PER_CORE * k + HP_CORE, :])}
        for k in range(NCORES)
    ]
    nc = _get_program()
    trace = bool(int(os.environ.get("MEDIAN_TRACE", "0")))
    res = run_bass_kernel_spmd(nc, in_maps, list(range(NCORES)), trace=trace)
    LAST_RESULT = res
    out = np.concatenate([res.results[k]["y"] for k in range(NCORES)], axis=1)
    return out.reshape(B, C, H, W).astype(np.float32)
